# revision 1
# baseline (speedup 1.0000x reference)
"""Trainium2 Bass kernel for nn_CrossEntropyLossWeight3.

Math: per row b of predict/target [B,16]:
  probs   = softmax(predict[b])
  pre     = argmax(predict[b]);  tar = argmax(target[b])
  w       = 0 if pre==tar else penalty[tar, pre]
  loss_b  = w * probs[pre]
out = mean_b(loss_b)

Key identities used on-device:
  probs[pre] = max(exp(x)) / sum(exp(x))          (softmax at its own argmax)
  penalty[i,j] = max(c_i,c_j)/(c_i+c_j) with distinct per-class counts c,
  so with u = c[pre], v = c[tar]:
  w = (u != v) * max(u,v)/(u+v)                   (diagonal -> 0 automatically)
  u, v are recovered with a one-hot dot product against the counts vector.

Sharding: pure data parallel over 8 cores (batch split); each core returns
per-partition partial sums [128,1]; host reduces and divides by B.
"""

import sys

sys.path.insert(0, "/opt/trn_rl_repo")

import numpy as np

import concourse.bass as bass
import concourse.bacc as bacc
import concourse.tile as tile
from concourse import mybir
from concourse.bass_utils import run_bass_kernel_spmd

B, W = 2097152, 16
NCORES = 8
BS = B // NCORES          # rows per core
P = 128                   # SBUF partitions
R = 256                   # rows per partition per tile
F = R * W                 # free elems per partition per tile
TILE_ROWS = P * R
NT = BS // TILE_ROWS      # tiles per core

LABELS_NUM_COUNT = [500000, 120000, 80000, 45000, 30000, 250000, 15000, 9000,
                    60000, 7000, 180000, 22000, 11000, 95000, 5000, 40000]

f32 = mybir.dt.float32
AX = mybir.AxisListType
OP = mybir.AluOpType
ACT = mybir.ActivationFunctionType


u32 = mybir.dt.uint32
PAYLOAD_BITS = 9          # counts/1000 <= 500 fits in 9 bits exactly
PAYLOAD_MASK = (1 << PAYLOAD_BITS) - 1
KEEP_MASK = 0xFFFFFFFF ^ PAYLOAD_MASK


def _register_custom_ops():
    """Runtime-registered custom DVE ops (single-pass fusions):

    EMBED_OXO_ANT:    out = ((in0 | c) ^ c) | in1, c = 0x1FF — clear the low
                      9 mantissa bits and OR in the per-class payload. The
                      OR/XOR form avoids an AND with 0xFFFFFE00, whose f32
                      bit pattern is a NaN and gets canonicalized to qNaN by
                      the scalar-operand path.
    EXTRACT_PAYF_ANT: out = float(bits(in0) & 0x1FF) via the 2^23 exponent
                      trick — payload bits to float value in one op.
    SELMAXNE_ANT:     out = (in0 != in1) ? max(in0, in1) : 0.
    """
    import numpy as np_

    from concourse.dve_spec import (
        Spec, Src0, Src1, C0, C1, Bin, AluOp, lower, select, ne, maxx, Zero,
    )
    from concourse.dve_ops import (
        DveOp,
        OPS,
        CUSTOM_DVE_SPECS,
        _SUB_OPCODE_FOR_NAME,
        _CUSTOM_DVE_ROW_BASE,
    )
    from concourse.dve_uop import DveOpSpec

    def reg(name, spec, rd1):
        for o in OPS:
            if o.name == name:
                return o
        shas = {}
        for ver in ("v3", "v4"):
            uops = lower(spec, ver=ver)
            s = DveOpSpec(
                name=name,
                opcode=_CUSTOM_DVE_ROW_BASE + len(OPS),
                uops=uops,
                rd1_en=rd1,
            )
            shas[ver] = s.sha(ver)
        op = DveOp(name, spec, subdim=False, uops_sha=shas)
        _SUB_OPCODE_FOR_NAME[name] = _CUSTOM_DVE_ROW_BASE + len(OPS)
        OPS.append(op)
        CUSTOM_DVE_SPECS[name] = spec
        return op

    # EMBMAX_SEG_ANT: one-pass fused "embed payload + segmented max".
    # body = Scan(MAX, embed_expr, _subdim_step=Zero) over a [P, S, 16] view;
    # the stock lowering treats subdim-scans as PageIdx-style hold counters,
    # so lower() runs with two patches:
    #   - steady state: d <- MAX(CURR, embed_expr)   (combine, not hold)
    #   - SUB_DIM_DONE step state: d <- BYPASS(embed_expr)  (restart segment)
    # With the out AP's innermost dim stride-0, the last write per segment
    # leaves the segment max directly in a [P, S] tile.
    import concourse.dve_spec as ds

    embed_expr = Bin(
        AluOp.BITWISE_OR,
        Bin(AluOp.BITWISE_XOR, Bin(AluOp.BITWISE_OR, Src0, C0), C0),
        Src1,
    )

    def _ref_embmax(in0, in1, s0, s1, imm2):
        emb = (
            ((in0.view(np_.uint32) | PAYLOAD_MASK) ^ PAYLOAD_MASK)
            | in1.view(np_.uint32)
        ).view(np_.float32)
        return np_.maximum.accumulate(emb, axis=-1)

    def reg_embmax():
        name = "EMBMAX_SEG_ANT"
        for o in OPS:
            if o.name == name:
                return o
        seg = ds.Scan(op=AluOp.MAX, expr=embed_expr, init=None, _subdim_step=Zero)
        spec = Spec(body=seg, reference=_ref_embmax)
        orig_so, orig_nas = ds._scan_overrides, ds._node_as_stage

        def patched_so(scans, node_stage):
            seed, step = {}, {}
            for scan in scans:
                d = node_stage[scan]
                init = (
                    scan.init
                    if scan.init is not None
                    else ds._ACCUM_IDENTITY[scan.op]
                )
                seed[d] = orig_nas(init)
                if scan._subdim_step is not None:
                    step[d] = ds._Stage(AluOp.BYPASS, scan.expr)
            return seed, step

        def patched_nas(e):
            if isinstance(e, ds.Scan) and e._subdim_step is not None:
                return ds._Stage(e.op, ds.AluInp.CURR_ALU_OUT, e.expr)
            return orig_nas(e)

        uops_by_ver, shas = {}, {}
        ds._scan_overrides, ds._node_as_stage = patched_so, patched_nas
        try:
            for ver in ("v3", "v4"):
                uops_by_ver[ver] = lower(spec, ver=ver)
        finally:
            ds._scan_overrides, ds._node_as_stage = orig_so, orig_nas
        opcode = _CUSTOM_DVE_ROW_BASE + len(OPS)
        from concourse.dve_ops import _COMPILE_CACHE

        for ver in ("v3", "v4"):
            s = DveOpSpec(name=name, opcode=opcode, uops=uops_by_ver[ver], rd1_en=True)
            shas[ver] = s.sha(ver)
            _COMPILE_CACHE[(name, ver)] = s
        op = DveOp(name, spec, subdim=True, uops_sha=shas)
        _SUB_OPCODE_FOR_NAME[name] = opcode
        OPS.append(op)
        CUSTOM_DVE_SPECS[name] = spec
        return op

    embed = reg_embmax()
    ext = reg(
        "EXTRACT_PAYF_ANT",
        Spec(
            body=Bin(
                AluOp.SUBTRACT,
                Bin(AluOp.BITWISE_OR, Bin(AluOp.BITWISE_AND, Src0, C0), C1),
                C1,
            ),
            reference=lambda in0, in1, s0, s1, imm2: (
                (
                    (in0.view(np_.uint32) & PAYLOAD_MASK)
                    | np_.float32(8388608.0).view(np_.uint32)
                ).view(np_.float32)
                - np_.float32(8388608.0)
            ),
        ),
        rd1=False,
    )
    smn = reg(
        "SELMAXNE_ANT",
        Spec(
            body=select(ne(Src0, Src1), maxx(Src0, Src1), Zero),
            reference=lambda in0, in1, s0, s1, imm2: np_.where(
                in0 != in1, np_.maximum(in0, in1), 0.0
            ).astype(np_.float32),
        ),
        rd1=True,
    )
    return embed, ext, smn


def _emit_big(nc, pools, pred_v, targ_v, pay_b, t, ops, mask_ap, me2, mt2, s2, half):
    """Streaming part for one [128, R*16] tile; row stats land in column
    block `half` of the [128, 2R] pair tiles."""
    io_pool, work_pool, small_pool = pools
    embed_op = ops[0]
    cols = slice(half * R, (half + 1) * R)

    xp = io_pool.tile([P, F], f32, tag="xp")
    nc.sync.dma_start(out=xp[:, :], in_=pred_v[t])
    xt = io_pool.tile([P, F], f32, tag="xt")
    nc.sync.dma_start(out=xt[:, :], in_=targ_v[t])

    # E = exp(predict) on ScalarE, in place over the loaded tile
    e = xp
    nc.scalar.activation(e[:, :], xp[:, :], ACT.Exp)
    e3 = e[:, :].rearrange("p (r w) -> p r w", w=W)

    # fused embed + segmented max in ONE DVE pass; stride-0 out leaves the
    # per-row max (with payload) directly in me2
    nc.vector._custom_dve(
        embed_op,
        out=me2[:, cols].unsqueeze(2).broadcast_to([P, R, W]),
        in0=e3, in1=pay_b, s0=mask_ap,
    )

    # row sums of (raw) E via pairwise-add tree on GPSIMD
    e4 = e[:, :].rearrange("p (r h two) -> p r h two", h=8, two=2)
    l1 = work_pool.tile([P, R * 8], f32, tag="l1")
    l1v = l1[:, :].rearrange("p (r h) -> p r h", h=8)
    nc.gpsimd.tensor_tensor(l1v, e4[:, :, :, 0], e4[:, :, :, 1], op=OP.add)
    l14 = l1[:, :].rearrange("p (r h two) -> p r h two", h=4, two=2)
    l2 = work_pool.tile([P, R * 4], f32, tag="l2")
    l2v = l2[:, :].rearrange("p (r h) -> p r h", h=4)
    nc.gpsimd.tensor_tensor(l2v, l14[:, :, :, 0], l14[:, :, :, 1], op=OP.add)
    l24 = l2[:, :].rearrange("p (r h two) -> p r h two", h=2, two=2)
    l3 = work_pool.tile([P, R * 2], f32, tag="l3")
    l3v = l3[:, :].rearrange("p (r h) -> p r h", h=2)
    nc.gpsimd.tensor_tensor(l3v, l24[:, :, :, 0], l24[:, :, :, 1], op=OP.add)
    l34 = l3[:, :].rearrange("p (r two) -> p r two", two=2)
    nc.gpsimd.tensor_tensor(
        s2[:, cols].unsqueeze(2), l34[:, :, 0:1], l34[:, :, 1:2], op=OP.add
    )

    # target side: fused embed + segmented max, same one-pass trick
    xt3 = xt[:, :].rearrange("p (r w) -> p r w", w=W)
    nc.vector._custom_dve(
        embed_op,
        out=mt2[:, cols].unsqueeze(2).broadcast_to([P, R, W]),
        in0=xt3, in1=pay_b, s0=mask_ap,
    )


def _emit_formula(nc, small_pool, acc, me2, mt2, s2, ops, mask_ap):
    """Per-row tail on a [128, 2R] batch:
    loss = (u!=v) * max(u,v) * clean(me) / ((u+v)*s)."""
    _, ext_op, smn_op = ops
    R2 = GRP * R
    u = small_pool.tile([P, R2], f32, tag="u")
    nc.vector._custom_dve(ext_op, out=u[:, :], in0=me2[:, :], s0=mask_ap,
                          s1=8388608.0)
    v = small_pool.tile([P, R2], f32, tag="v")
    nc.vector._custom_dve(ext_op, out=v[:, :], in0=mt2[:, :], s0=mask_ap,
                          s1=8388608.0)
    num = small_pool.tile([P, R2], f32, tag="num")
    nc.vector._custom_dve(smn_op, out=num[:, :], in0=u[:, :], in1=v[:, :])
    # me2 still carries payload bits in the low mantissa: <= 2^-14 relative
    # perturbation of max(E), well inside the accuracy budget.
    # The three stock multiplies/adds run on GPSIMD to keep DVE free.
    nc.gpsimd.tensor_tensor(num[:, :], num[:, :], me2[:, :], op=OP.mult)
    den = small_pool.tile([P, R2], f32, tag="den")
    nc.gpsimd.tensor_tensor(den[:, :], u[:, :], v[:, :], op=OP.add)
    nc.gpsimd.tensor_tensor(den[:, :], den[:, :], s2[:, :], op=OP.mult)
    rec = small_pool.tile([P, R2], f32, tag="rec")
    nc.vector.reciprocal_approx_fast(out=rec[:, :], in_=den[:, :])
    nc.vector.tensor_tensor(num[:, :], num[:, :], rec[:, :], op=OP.mult)
    tsum = small_pool.tile([P, 1], f32, tag="tsum")
    nc.vector.reduce_sum(tsum[:, :], num[:, :], axis=AX.X)
    nc.vector.tensor_tensor(acc[:, :], acc[:, :], tsum[:, :], op=OP.add)


GRP = 4


def _emit_pair(nc, pools, pred_v, targ_v, pay_b, acc, pair, ops, mask_ap):
    small_pool = pools[2]
    me2 = small_pool.tile([P, GRP * R], f32, tag="me")
    mt2 = small_pool.tile([P, GRP * R], f32, tag="mt")
    s2 = small_pool.tile([P, GRP * R], f32, tag="s")
    for half in range(GRP):
        t = pair * GRP + half
        _emit_big(nc, pools, pred_v, targ_v, pay_b, t, ops, mask_ap,
                  me2, mt2, s2, half)
    _emit_formula(nc, small_pool, acc, me2, mt2, s2, ops, mask_ap)


def _build_program(passes=1, dyn_iters=False):
    """v2: bit-packed class weights. counts/1000 (exact, 9 bits) are OR'd
    into the low mantissa bits of exp(predict) / target, so a single grouped
    reduce_max per side yields both the row max and the argmax's class
    weight. Perturbation is <= 2^-14 relative.

    dyn_iters=True wraps the tile loop in a runtime-count For_i (bound read
    from a `niter` input tensor) — used only for timing with a fixed NEFF."""
    nc = bacc.Bacc("TRN2", target_bir_lowering=False, debug=False)
    pred = nc.dram_tensor("predict", [BS, W], f32, kind="ExternalInput")
    targ = nc.dram_tensor("target", [BS, W], f32, kind="ExternalInput")
    pay = nc.dram_tensor("payload", [P, W], u32, kind="ExternalInput")
    if dyn_iters:
        nit = nc.dram_tensor("niter", [1, 1], mybir.dt.int32, kind="ExternalInput")
    out = nc.dram_tensor("out", [P, 1], f32, kind="ExternalOutput")

    pred_v = pred[:, :].rearrange("(t p r) w -> t p (r w)", t=NT, p=P, r=R)
    targ_v = targ[:, :].rearrange("(t p r) w -> t p (r w)", t=NT, p=P, r=R)

    with tile.TileContext(nc) as tc:
        with (
            tc.tile_pool(name="io", bufs=2) as io_pool,
            tc.tile_pool(name="work", bufs=2) as work_pool,
            tc.tile_pool(name="small", bufs=2) as small_pool,
            tc.tile_pool(name="const", bufs=1) as const_pool,
        ):
            pay_t = const_pool.tile([P, W], u32, tag="pay")
            nc.gpsimd.dma_start(out=pay_t[:, :], in_=pay[:, :])
            pay_b = pay_t[:, :].unsqueeze(1).broadcast_to([P, R, W]).bitcast(f32)

            mask_t = const_pool.tile([P, 1], u32, tag="mask")
            nc.vector.memset(mask_t[:, :], PAYLOAD_MASK)
            mask_ap = mask_t[:, :1].bitcast(f32)

            acc = const_pool.tile([P, 1], f32, tag="acc")
            nc.vector.memset(acc[:, :], 0.0)

            ops = _register_custom_ops()
            pools = (io_pool, work_pool, small_pool)
            if dyn_iters:
                from concourse.bass import RegisterHandles, make_scalar_value

                nit_t = const_pool.tile([1, 1], mybir.dt.int32, tag="nit")
                nc.gpsimd.dma_start(out=nit_t[:, :], in_=nit[:, :])
                regs = []
                for ename, eng in nc.engines.items():
                    r = eng.alloc_register(f"nit_{ename}")
                    eng.reg_load(r, nit_t[:1, :1])
                    regs.append(r)
                n = make_scalar_value(RegisterHandles(regs), min_val=1, max_val=1024)
                with tc.For_i(0, n, 1):
                    for pair in range(NT // GRP):
                        _emit_pair(nc, pools, pred_v, targ_v, pay_b, acc, pair, ops, mask_ap)
            else:
                for pair in range((NT // GRP) * passes):
                    _emit_pair(nc, pools, pred_v, targ_v, pay_b, acc, pair % (NT // GRP), ops, mask_ap)

            nc.sync.dma_start(out=out[:, :], in_=acc[:, :])
    nc.compile()
    return nc


def _build_program_v1(passes=1):
    nc = bacc.Bacc("TRN2", target_bir_lowering=False, debug=False)
    pred = nc.dram_tensor("predict", [BS, W], f32, kind="ExternalInput")
    targ = nc.dram_tensor("target", [BS, W], f32, kind="ExternalInput")
    cnts = nc.dram_tensor("counts", [P, W], f32, kind="ExternalInput")
    out = nc.dram_tensor("out", [P, 1], f32, kind="ExternalOutput")

    pred_v = pred[:, :].rearrange("(t p r) w -> t p (r w)", t=NT, p=P, r=R)
    targ_v = targ[:, :].rearrange("(t p r) w -> t p (r w)", t=NT, p=P, r=R)

    with tile.TileContext(nc) as tc:
        with (
            tc.tile_pool(name="io", bufs=2) as io_pool,
            tc.tile_pool(name="work", bufs=3) as work_pool,
            tc.tile_pool(name="small", bufs=2) as small_pool,
            tc.tile_pool(name="const", bufs=1) as const_pool,
        ):
            counts_t = const_pool.tile([P, W], f32, tag="counts")
            nc.gpsimd.dma_start(out=counts_t[:, :], in_=cnts[:, :])
            counts_b = counts_t[:, :].unsqueeze(1).broadcast_to([P, R, W])

            acc = const_pool.tile([P, 1], f32, tag="acc")
            nc.vector.memset(acc[:, :], 0.0)

            for t in range(NT * passes):
                t = t % NT
                xp = io_pool.tile([P, F], f32, tag="xp")
                nc.gpsimd.dma_start(out=xp[:, :], in_=pred_v[t])
                xt = io_pool.tile([P, F], f32, tag="xt")
                nc.gpsimd.dma_start(out=xt[:, :], in_=targ_v[t])
                xp3 = xp[:, :].rearrange("p (r w) -> p r w", w=W)
                xt3 = xt[:, :].rearrange("p (r w) -> p r w", w=W)

                # E = exp(predict) on ScalarE
                e = work_pool.tile([P, F], f32, tag="e")
                nc.scalar.activation(e[:, :], xp[:, :], ACT.Exp)
                e3 = e[:, :].rearrange("p (r w) -> p r w", w=W)

                # row max / row sum of E
                me = small_pool.tile([P, R], f32, tag="me")
                nc.vector.reduce_max(me[:, :], e3, axis=AX.X)
                s = small_pool.tile([P, R], f32, tag="s")
                nc.vector.reduce_sum(s[:, :], e3, axis=AX.X)
                me_b = me[:, :].unsqueeze(2).broadcast_to([P, R, W])

                # one-hot(argmax(predict)) . counts  -> u  (reuse e tile)
                nc.vector.tensor_tensor(e3, e3, me_b, op=OP.is_equal)
                nc.vector.tensor_tensor(e3, e3, counts_b, op=OP.mult)
                u = small_pool.tile([P, R], f32, tag="u")
                nc.vector.reduce_sum(u[:, :], e3, axis=AX.X)

                # one-hot(argmax(target)) . counts -> v  (in-place on xt)
                mt = small_pool.tile([P, R], f32, tag="mt")
                nc.vector.reduce_max(mt[:, :], xt3, axis=AX.X)
                mt_b = mt[:, :].unsqueeze(2).broadcast_to([P, R, W])
                nc.vector.tensor_tensor(xt3, xt3, mt_b, op=OP.is_equal)
                nc.vector.tensor_tensor(xt3, xt3, counts_b, op=OP.mult)
                v = small_pool.tile([P, R], f32, tag="v")
                nc.vector.reduce_sum(v[:, :], xt3, axis=AX.X)

                # loss = (u!=v) * max(u,v)*me / ((u+v)*s)
                mx = small_pool.tile([P, R], f32, tag="mx")
                nc.vector.tensor_tensor(mx[:, :], u[:, :], v[:, :], op=OP.max)
                sm = small_pool.tile([P, R], f32, tag="sm")
                nc.vector.tensor_tensor(sm[:, :], u[:, :], v[:, :], op=OP.add)
                ne = small_pool.tile([P, R], f32, tag="ne")
                nc.vector.tensor_tensor(ne[:, :], u[:, :], v[:, :], op=OP.not_equal)
                nc.vector.tensor_tensor(mx[:, :], mx[:, :], ne[:, :], op=OP.mult)
                nc.vector.tensor_tensor(mx[:, :], mx[:, :], me[:, :], op=OP.mult)
                nc.vector.tensor_tensor(sm[:, :], sm[:, :], s[:, :], op=OP.mult)
                rec = small_pool.tile([P, R], f32, tag="rec")
                nc.vector.reciprocal_approx_fast(out=rec[:, :], in_=sm[:, :])
                nc.vector.tensor_tensor(mx[:, :], mx[:, :], rec[:, :], op=OP.mult)

                tsum = small_pool.tile([P, 1], f32, tag="tsum")
                nc.vector.reduce_sum(tsum[:, :], mx[:, :], axis=AX.X)
                nc.vector.tensor_tensor(acc[:, :], acc[:, :], tsum[:, :], op=OP.add)

            nc.sync.dma_start(out=out[:, :], in_=acc[:, :])
    nc.compile()
    return nc


def _fix_dma_waits(nc):
    """Walrus codegen only supports ONE sync-wait on DMA instructions
    (NEURON_ISA_TPB_EVENTS has a single wait slot). The Tile scheduler can
    emit two: a compute-engine WAR wait plus a DMA-queue credit wait. The
    credit wait is transitively implied by the engine wait here: the engine
    that read/produced the slot's previous contents had itself waited on the
    previous DMA's completion semaphore. Drop the DMA-queue waits whenever a
    compute-engine wait remains."""
    for bb in nc.main_func.blocks:
        for ins in bb.instructions:
            if type(ins).__name__ != "InstDMACopy":
                continue
            si = ins.sync_info
            if si is None or len(si.on_wait) <= 1:
                continue
            keep = [
                w
                for w in si.on_wait
                if not (w.ant_name or "").startswith(("DMAHW", "DMASW"))
            ]
            assert len(keep) == 1, (
                f"{ins.name}: cannot reduce waits to one: "
                f"{[(w.ant_name, w.wait_value) for w in si.on_wait]}"
            )
            si.on_wait = keep
            ins.sync_info = si


_CACHE = {}


def _run(predict, target, trace=False):
    if "nc" not in _CACHE:
        _CACHE["nc"] = _build_program()
    nc = _CACHE["nc"]

    predict = np.ascontiguousarray(np.asarray(predict, dtype=np.float32))
    target = np.ascontiguousarray(np.asarray(target, dtype=np.float32))
    payload = np.broadcast_to(
        (np.asarray(LABELS_NUM_COUNT, dtype=np.uint32) // 1000)[None, :], (P, W)
    ).copy()

    in_maps = []
    for i in range(NCORES):
        in_maps.append(
            {
                "predict": predict[i * BS : (i + 1) * BS],
                "target": target[i * BS : (i + 1) * BS],
                "payload": payload,
            }
        )
    res = run_bass_kernel_spmd(nc, in_maps, core_ids=list(range(NCORES)), trace=trace)
    total = np.float64(0.0)
    for r in res.results:
        total += np.float64(r["out"].astype(np.float64).sum())
    value = np.float32(total / B)
    return np.asarray(value, dtype=np.float32), res


def kernel(predict, target, penalty_matrix=None):
    value, _ = _run(predict, target, trace=False)
    return value

